# revision 8
# baseline (speedup 1.0000x reference)
"""Causal self-attention (B=2, S=2048, D=2048, H=16, HD=128) on 8 TRN2 cores.

Sharding: core c -> batch b = c//4, heads 4*(c%4)..4*(c%4)+3 (tensor-parallel
over heads within a batch; data-parallel over batch across core groups).

v4 design (v3 ~400us measured):
  - RoPE via host-side de-interleave permutation of Wq/Wk columns (evens then
    odds per head) + permuted cos/sin tables with the rotate sign folded into
    sin. rotate_half becomes a partition-half swap, done by reading the
    projection PSUM at partition offset 64 in the DVE multiplies. Kills the
    pmat matmul and the qraw ACT evacuation of v3. Scores are invariant to the
    permutation (q,k both permuted); V path untouched.
  - head-priority DMA: V projections run first (no RoPE tables needed), with
    x0/wv quarters leading the scalar/sync rings and the (bf16) cos/sin
    tables on the vector ring. v3 put 2MB of f32 tables ahead of x0 which
    delayed the first matmul to 39us.
  - softmax row-sums fully off the PE: DVE/gpsimd accumulate exp chunks, one
    ones-matrix matmul per window reduces to a *replicated* [128,512] row-sum
    (M=128 costs the same as M=1), so the reciprocal needs no gpsimd
    partition_broadcast. reciprocal_approx_fast only (~18 bits, output is
    bf16 anyway).
  - qb-major window order: output-projection sub-jobs (4 matmuls each) become
    available 4 windows in and are dripped one per chunk, keeping the PE
    continuously fed (the PE clocks down after any idle and needs 3us of
    continuous execution to re-reach max p-state).
  - eager PV evacuation on ACT; outproj evacuation split DVE/gpsimd; mask
    multiplies + diagonal-chunk accumulation on gpsimd to keep every engine
    under ~70%.
"""

import math
from collections import deque

import ml_dtypes
import numpy as np

import concourse.bacc as bacc
import concourse.mybir as mybir
from concourse.tile import TileContext
from concourse.bass_utils import run_bass_kernel_spmd

B, S, D = 2, 2048, 2048
H, HD = 16, 128
ROPE_THETA = 10000.0

N_CORES = 8
CORES_PER_BATCH = 4
HPC = H // (N_CORES // B)  # heads per core = 4
HL = HPC * HD              # 512 local head-dim columns
NDC = D // 128             # 16 contraction chunks
NSB = S // 512             # 4 s-blocks
NKC = S // 128             # 16 k-chunks

F32 = mybir.dt.float32
BF16 = mybir.dt.bfloat16
AF = mybir.ActivationFunctionType
BNP = ml_dtypes.bfloat16

FIN_LAG = 3   # projection finishers lag this many groups
PV_LAG = 2    # PV matmuls lag score matmuls by this many chunks


def _mm(nc, out, lhsT, rhs, start, stop):
    nc.tensor.matmul(out, lhsT, rhs, start=start, stop=stop)


def _build():
    nc = bacc.Bacc("TRN2", target_bir_lowering=False, debug=False)

    # all big operands arrive pre-rearranged to the on-chip layout so each
    # DMA is one contiguous run per partition
    xT = nc.dram_tensor("xT", [128, NSB, NDC, 512], BF16, kind="ExternalInput")
    wq = nc.dram_tensor("wq", [128, NDC, HL], BF16, kind="ExternalInput")
    wk = nc.dram_tensor("wk", [128, NDC, HL], BF16, kind="ExternalInput")
    wv = nc.dram_tensor("wv", [128, NDC, HL], BF16, kind="ExternalInput")
    wo = nc.dram_tensor("wo", [128, HL // 128, D], BF16, kind="ExternalInput")
    cosP = nc.dram_tensor("cosP", [HD, S], BF16, kind="ExternalInput")
    sinP = nc.dram_tensor("sinP", [HD, S], BF16, kind="ExternalInput")
    maskT = nc.dram_tensor("maskT", [128, 512], BF16, kind="ExternalInput")
    onesd = nc.dram_tensor("onesd", [128, 128], BF16, kind="ExternalInput")
    out = nc.dram_tensor("out", [S, D], BF16, kind="ExternalOutput")

    with TileContext(nc) as tc:
        with (
            tc.tile_pool(name="consts", bufs=1) as consts,
            tc.tile_pool(name="resid", bufs=1) as resid,
        ):
            # SBUF-resident q^T/k^T (per head, RoPE'd+permuted) and v, bf16
            q_sb = [resid.tile([HD, S], BF16, name=f"qT{h}") for h in range(HPC)]
            k_sb = [resid.tile([HD, S], BF16, name=f"kT{h}") for h in range(HPC)]
            v_sb = resid.tile([128, NKC, HL], BF16, name="v_sb")
            ctxs = [resid.tile([128, S], BF16, name=f"ctxT{h}") for h in range(HPC)]

            cos_sb = consts.tile([HD, S], BF16, name="cos_sb")
            sin_sb = consts.tile([HD, S], BF16, name="sin_sb")
            mask_sb = consts.tile([128, 512], BF16, name="mask_sb")
            ones_sb = consts.tile([128, 128], BF16, name="ones_sb")
            gpwarm = consts.tile([128, 128], F32, name="gpwarm")

            # ---------------- phase 1: projections + RoPE ----------------
            with (
                tc.tile_pool(name="wpool", bufs=1) as wpool,
                tc.tile_pool(name="xtp", bufs=2) as xtp,
                tc.tile_pool(name="st1", bufs=2) as st1,
                tc.tile_pool(name="psP", bufs=5, space="PSUM") as psP,
            ):
                w_sb = {}
                for nm in ("wq", "wk", "wv"):
                    w_sb[nm] = wpool.tile([128, NDC, HL], BF16, name=f"{nm}_sb")

                # ring priority: the first-needed bytes lead each queue
                # (only sync/scalar/gpsimd can issue DMAs, and gpsimd's
                # library load stalls its queue ~11us at boot).
                # sync: wv quarters, wq quarters, wk. scalar: x0 quarters,
                # RoPE tables, x1. gpsimd: warmup, then x2/x3.
                for qt in range(4):
                    nc.sync.dma_start(
                        out=w_sb["wv"][:, qt * 4:(qt + 1) * 4, :],
                        in_=wv[:, qt * 4:(qt + 1) * 4, :])

                def load_x_block(sb, quartered=False, eng=None):
                    eng = eng or nc.scalar
                    xt = xtp.tile([128, NDC, 512], BF16, tag="xt", name="xt")
                    if quartered:
                        for qt in range(4):
                            eng.dma_start(
                                out=xt[:, qt * 4:(qt + 1) * 4, :],
                                in_=xT[:, sb, qt * 4:(qt + 1) * 4, :])
                    else:
                        eng.dma_start(out=xt[:], in_=xT[:, sb])
                    return xt

                x_cur = load_x_block(0, quartered=True)
                for qt in range(4):
                    nc.sync.dma_start(
                        out=w_sb["wq"][:, qt * 4:(qt + 1) * 4, :],
                        in_=wq[:, qt * 4:(qt + 1) * 4, :])
                nc.scalar.dma_start(out=cos_sb[:], in_=cosP[:])
                nc.scalar.dma_start(out=sin_sb[:], in_=sinP[:])
                nc.scalar.dma_start(out=mask_sb[:], in_=maskT[:])
                nc.scalar.dma_start(out=ones_sb[:], in_=onesd[:])
                nc.sync.dma_start(out=w_sb["wk"][:], in_=wk[:])
                x_next = load_x_block(1)
                nc.gpsimd.partition_broadcast(gpwarm[:], gpwarm[0:1, :])

                finishers = deque()

                def emit_finisher():
                    kind, args = finishers.popleft()
                    if kind == "qk":
                        ps, dst, sl = args
                        ta = st1.tile([128, 512], F32, tag="ta", name="ta")
                        nc.vector.tensor_mul(ta[:], ps[:], cos_sb[:, sl])
                        tb = st1.tile([128, 512], F32, tag="tb", name="tb")
                        # rotate_half in permuted space = swap 64-halves;
                        # the sign lives in sin_sb (host-folded)
                        nc.vector.tensor_mul(tb[0:64, :], ps[64:128, :],
                                             sin_sb[0:64, sl])
                        nc.vector.tensor_mul(tb[64:128, :], ps[0:64, :],
                                             sin_sb[64:128, sl])
                        nc.vector.tensor_add(dst[:, sl], ta[:], tb[:])
                    else:
                        ps, kc = args
                        nc.scalar.activation(v_sb[:, kc, :], ps[:], AF.Copy)

                for sb in range(NSB):
                    sl = slice(sb * 512, (sb + 1) * 512)
                    if sb > 0:
                        x_cur = x_next
                        if sb < NSB - 1:
                            x_next = load_x_block(sb + 1, eng=nc.gpsimd)

                    # V first: no table dependency, so the RoPE tables get
                    # ~14us of slack on the vector ring
                    for sc in range(4):
                        ps = psP.tile([128, 512], F32, tag="p", name="ps")
                        for dc in range(NDC):
                            _mm(nc, ps[:],
                                x_cur[:, dc, sc * 128:(sc + 1) * 128],
                                w_sb["wv"][:, dc, :],
                                start=(dc == 0), stop=(dc == NDC - 1))
                        finishers.append(("v", (ps, sb * 4 + sc)))
                        if len(finishers) > FIN_LAG:
                            emit_finisher()

                    for wname, dst in (("wq", q_sb), ("wk", k_sb)):
                        w_t = w_sb[wname]
                        for h in range(HPC):
                            ps = psP.tile([128, 512], F32, tag="p", name="ps")
                            for dc in range(NDC):
                                _mm(nc, ps[:],
                                    w_t[:, dc, h * HD:(h + 1) * HD],
                                    x_cur[:, dc, :],
                                    start=(dc == 0), stop=(dc == NDC - 1))
                            finishers.append(("qk", (ps, dst[h], sl)))
                            if len(finishers) > FIN_LAG:
                                emit_finisher()
                while finishers:
                    emit_finisher()

            # ---------- phase 2+3: attention + output projection ----------
            with (
                tc.tile_pool(name="pp", bufs=6) as pp,
                tc.tile_pool(name="accp", bufs=3) as accp,
                tc.tile_pool(name="sm", bufs=2) as sm,
                tc.tile_pool(name="pvp", bufs=3) as pvp,
                tc.tile_pool(name="wop", bufs=1) as wop,
                tc.tile_pool(name="outp", bufs=4) as outp,
                tc.tile_pool(name="psA", bufs=3, space="PSUM") as psA,
                tc.tile_pool(name="psB", bufs=2, space="PSUM") as psB,
                tc.tile_pool(name="psC", bufs=1, space="PSUM") as psC,
                tc.tile_pool(name="psD", bufs=2, space="PSUM") as psD,
            ):
                wo_sb = wop.tile([128, HPC * D], BF16, name="wo_sb")
                nc.sync.dma_start(out=wo_sb[:], in_=wo[:])

                lagq = deque()
                Oq = deque()          # outproj sub-jobs (qc, db)
                done_cnt = [0] * NSB  # finalized heads per q-block

                def emit_lpv(job):
                    pv, pt, vtc, ncols, first, last, w = job
                    _mm(nc, pv[:, 512 - ncols:], vtc, pt[:, :ncols],
                        start=first, stop=last)
                    if last:
                        # eager PSUM evacuation (ACT) so the pv bank recycles
                        pvs = pvp.tile([128, 512], BF16, tag="pvs", name="pvs")
                        nc.scalar.activation(pvs[:], pv[:], AF.Copy)
                        w["pvs"] = pvs

                def emit_chain_a(w):
                    # replicated row-sum: ones[128,128]^T @ acc -> every
                    # partition holds l, so no partition_broadcast needed
                    lps = psC.tile([128, 512], F32, tag="c", name="lps")
                    _mm(nc, lps[:], ones_sb[:], w["acc"][:],
                        start=True, stop=True)
                    rcp = sm.tile([128, 512], F32, tag="rcp", name="rcp")
                    nc.vector.reciprocal_approx_fast(rcp[:], lps[:])
                    w["rcp"] = rcp

                def emit_chain_b(w):
                    qb = w["qb"]
                    # SBUF-only operands, so this can live on gpsimd
                    nc.gpsimd.tensor_mul(
                        ctxs[w["h"]][:, qb * 512:(qb + 1) * 512],
                        w["pvs"][:], w["rcp"][:])
                    done_cnt[qb] += 1
                    if done_cnt[qb] == HPC:
                        for qc in range(4 * qb, 4 * qb + 4):
                            for db in range(D // 512):
                                Oq.append((qc, db))

                def emit_outproj():
                    qc, db = Oq.popleft()
                    ops = psD.tile([128, 512], F32, tag="d", name="ops")
                    for hh in range(HPC):
                        _mm(nc, ops[:],
                            ctxs[hh][:, qc * 128:(qc + 1) * 128],
                            wo_sb[:, hh * D + db * 512:hh * D + (db + 1) * 512],
                            start=(hh == 0), stop=(hh == HPC - 1))
                    osb = outp.tile([128, 512], BF16, tag="osb", name="osb")
                    # gpsimd cannot read PSUM; all outproj evacuations on DVE
                    nc.vector.tensor_copy(osb[:], ops[:])
                    nc.sync.dma_start(
                        out=out[qc * 128:(qc + 1) * 128,
                                db * 512:(db + 1) * 512],
                        in_=osb[:])

                prev_w = None
                for qb in range(NSB):
                    for h in range(HPC):
                        nk = 4 * qb + 4
                        w = {"h": h, "qb": qb}
                        pv = psB.tile([128, 512], F32, tag="b", name="pv")
                        acc = accp.tile([128, 512], BF16, tag="acc", name="acc")
                        w["acc"] = acc
                        for kc in range(nk):
                            j = kc - 4 * qb
                            ncols = 512 if j < 0 else 512 - 128 * j
                            sps = psA.tile([128, 512], F32, tag="a", name="sps")
                            _mm(nc, sps[:, :ncols],
                                k_sb[h][:, kc * 128:(kc + 1) * 128],
                                q_sb[h][:, qb * 512 + 512 - ncols:(qb + 1) * 512],
                                start=True, stop=True)
                            pt = pp.tile([128, 512], BF16, tag="pt", name="pt")
                            nc.scalar.activation(pt[:, :ncols], sps[:, :ncols],
                                                 AF.Exp)
                            diag = j >= 0
                            eng = nc.gpsimd if diag else nc.vector
                            if diag:
                                nc.gpsimd.tensor_mul(pt[:, :ncols],
                                                     pt[:, :ncols],
                                                     mask_sb[:, :ncols])
                            if kc == 0:
                                eng.tensor_copy(acc[:], pt[:])
                            else:
                                eng.tensor_add(acc[:, 512 - ncols:],
                                               acc[:, 512 - ncols:],
                                               pt[:, :ncols])
                            lagq.append((pv, pt,
                                         v_sb[:, kc, h * HD:(h + 1) * HD],
                                         ncols, kc == 0, kc == nk - 1, w))
                            while len(lagq) > PV_LAG:
                                emit_lpv(lagq.popleft())
                            # one side action per chunk; the chain lags one
                            # extra chunk so the PE's l-reduce never waits on
                            # the accumulation engines
                            if kc == 1 and prev_w is not None:
                                emit_chain_a(prev_w)
                            elif kc == 2 and prev_w is not None:
                                emit_chain_b(prev_w)
                            elif Oq:
                                emit_outproj()
                        prev_w = w
                while lagq:
                    emit_lpv(lagq.popleft())
                emit_chain_a(prev_w)
                emit_chain_b(prev_w)
                while Oq:
                    emit_outproj()

    nc.compile()
    return nc


_NC_CACHE = None


def _get_nc():
    global _NC_CACHE
    if _NC_CACHE is None:
        _NC_CACHE = _build()
    return _NC_CACHE


# de-interleave: evens then odds, per head
_PERM = np.concatenate([np.arange(0, HD, 2), np.arange(1, HD, 2)])


def _host_tables():
    # Replicate reference RoPE tables in float32 arithmetic, permuted.
    inv_freq = np.float32(1.0) / np.power(
        np.float32(ROPE_THETA), np.arange(0, HD, 2).astype(np.float32) / np.float32(HD)
    )
    pos = np.arange(S, dtype=np.float32)
    freqs = pos[:, None] * inv_freq[None, :]
    angles = np.concatenate([freqs, freqs], axis=1)  # [S, HD]
    cos = np.cos(angles).astype(np.float32)
    sin = np.sin(angles).astype(np.float32)
    cos_p = np.ascontiguousarray(cos[:, _PERM].T)  # [HD, S]
    sin_p = np.ascontiguousarray(sin[:, _PERM].T).copy()
    sin_p[:HD // 2] *= np.float32(-1.0)  # fold rotate_half's sign
    mask = (np.arange(128)[:, None] <= np.arange(512)[None, :]).astype(BNP)
    return cos_p.astype(BNP), sin_p.astype(BNP), mask


_ONES = np.ones((128, 128), dtype=BNP)


def kernel(x, Wq, Wk, Wv, Wo):
    x = np.asarray(x, dtype=np.float32)
    Wq = np.asarray(Wq, dtype=np.float32)
    Wk = np.asarray(Wk, dtype=np.float32)
    Wv = np.asarray(Wv, dtype=np.float32)
    Wo = np.asarray(Wo, dtype=np.float32)

    results = _run_device(x, Wq, Wk, Wv, Wo)

    out = np.empty((B, S, D), dtype=np.float32)
    for b in range(B):
        acc = results[b * CORES_PER_BATCH]["out"].astype(np.float32)
        for i in range(1, CORES_PER_BATCH):
            acc = acc + results[b * CORES_PER_BATCH + i]["out"].astype(np.float32)
        out[b] = acc
    return out


def _make_in_maps(x, Wq, Wk, Wv, Wo):
    cos_p, sin_p, mask = _host_tables()
    scale = np.float32(1.0 / math.sqrt(HD))
    # permutation of a 512-col (4-head) slice: de-interleave within each head
    block_perm = np.concatenate([hh * HD + _PERM for hh in range(HPC)])

    def dev_w(w):  # [D, HL_slice] -> [128, NDC, hl]
        return np.ascontiguousarray(
            w.reshape(NDC, 128, -1).transpose(1, 0, 2)).astype(BNP)

    wq_scaled = (Wq * scale).astype(np.float32)
    xTb = [
        np.ascontiguousarray(
            x[b].T.reshape(NDC, 128, NSB, 512).transpose(1, 2, 0, 3)).astype(BNP)
        for b in range(B)
    ]
    in_maps = []
    for c in range(N_CORES):
        b = c // CORES_PER_BATCH
        g = c % CORES_PER_BATCH
        hs = slice(g * HL, (g + 1) * HL)
        in_maps.append({
            "xT": xTb[b],
            "wq": dev_w(wq_scaled[:, hs][:, block_perm]),
            "wk": dev_w(Wk[:, hs][:, block_perm]),
            "wv": dev_w(Wv[:, hs]),
            "wo": np.ascontiguousarray(
                Wo[hs, :].reshape(HL // 128, 128, D).transpose(1, 0, 2)).astype(BNP),
            "cosP": cos_p,
            "sinP": sin_p,
            "maskT": mask,
            "onesd": _ONES,
        })
    return in_maps


def _run_device(x, Wq, Wk, Wv, Wo, trace=False):
    nc = _get_nc()
    in_maps = _make_in_maps(x, Wq, Wk, Wv, Wo)
    res = run_bass_kernel_spmd(nc, in_maps, core_ids=list(range(N_CORES)), trace=trace)
    if trace:
        return res
    return res.results


def run_traced(x, Wq, Wk, Wv, Wo):
    """Run with NTFF tracing; returns (full_output, BassKernelResults)."""
    res = _run_device(np.asarray(x, np.float32), np.asarray(Wq, np.float32),
                      np.asarray(Wk, np.float32), np.asarray(Wv, np.float32),
                      np.asarray(Wo, np.float32), trace=True)
    out = np.empty((B, S, D), dtype=np.float32)
    for b in range(B):
        acc = res.results[b * CORES_PER_BATCH]["out"].astype(np.float32)
        for i in range(1, CORES_PER_BATCH):
            acc = acc + res.results[b * CORES_PER_BATCH + i]["out"].astype(np.float32)
        out[b] = acc
    return out, res


# revision 14
# speedup vs baseline: 1.2286x; 1.2286x over previous
"""Causal self-attention (B=2, S=2048, D=2048, H=16, HD=128) on 8 TRN2 cores.

Sharding: core c -> batch b = c//4, heads 4*(c%4)..4*(c%4)+3 (tensor-parallel
over heads within a batch; data-parallel over batch across core groups).

v4 design (v3 ~400us measured):
  - RoPE via host-side de-interleave permutation of Wq/Wk columns (evens then
    odds per head) + permuted cos/sin tables with the rotate sign folded into
    sin. rotate_half becomes a partition-half swap, done by reading the
    projection PSUM at partition offset 64 in the DVE multiplies. Kills the
    pmat matmul and the qraw ACT evacuation of v3. Scores are invariant to the
    permutation (q,k both permuted); V path untouched.
  - head-priority DMA: V projections run first (no RoPE tables needed), with
    x0/wv quarters leading the scalar/sync rings and the (bf16) cos/sin
    tables on the vector ring. v3 put 2MB of f32 tables ahead of x0 which
    delayed the first matmul to 39us.
  - softmax row-sums fully off the PE: DVE/gpsimd accumulate exp chunks, one
    ones-matrix matmul per window reduces to a *replicated* [128,512] row-sum
    (M=128 costs the same as M=1), so the reciprocal needs no gpsimd
    partition_broadcast. reciprocal_approx_fast only (~18 bits, output is
    bf16 anyway).
  - qb-major window order: output-projection sub-jobs (4 matmuls each) become
    available 4 windows in and are dripped one per chunk, keeping the PE
    continuously fed (the PE clocks down after any idle and needs 3us of
    continuous execution to re-reach max p-state).
  - eager PV evacuation on ACT; outproj evacuation split DVE/gpsimd; mask
    multiplies + diagonal-chunk accumulation on gpsimd to keep every engine
    under ~70%.
"""

import math
from collections import deque

import ml_dtypes
import numpy as np

import concourse.bacc as bacc
import concourse.mybir as mybir
from concourse.tile import TileContext
from concourse.bass_utils import run_bass_kernel_spmd

B, S, D = 2, 2048, 2048
H, HD = 16, 128
ROPE_THETA = 10000.0

N_CORES = 8
CORES_PER_BATCH = 4
HPC = H // (N_CORES // B)  # heads per core = 4
HL = HPC * HD              # 512 local head-dim columns
NDC = D // 128             # 16 contraction chunks
NSB = S // 512             # 4 s-blocks
NKC = S // 128             # 16 k-chunks

F32 = mybir.dt.float32
BF16 = mybir.dt.bfloat16
AF = mybir.ActivationFunctionType
BNP = ml_dtypes.bfloat16

FIN_LAG = 3   # projection finishers lag this many groups
PV_LAG = 3    # PV matmuls lag score matmuls by this many chunks


def _mm(nc, out, lhsT, rhs, start, stop):
    nc.tensor.matmul(out, lhsT, rhs, start=start, stop=stop)


def _build():
    nc = bacc.Bacc("TRN2", target_bir_lowering=False, debug=False)

    # all big operands arrive pre-rearranged to the on-chip layout so each
    # DMA is one contiguous run per partition
    xT = nc.dram_tensor("xT", [128, NSB, NDC, 512], BF16, kind="ExternalInput")
    wq = nc.dram_tensor("wq", [128, NDC, HL], BF16, kind="ExternalInput")
    wk = nc.dram_tensor("wk", [128, NDC, HL], BF16, kind="ExternalInput")
    wv = nc.dram_tensor("wv", [128, NDC, HL], BF16, kind="ExternalInput")
    wo = nc.dram_tensor("wo", [128, HL // 128, D], BF16, kind="ExternalInput")
    cosP = nc.dram_tensor("cosP", [HD, S], BF16, kind="ExternalInput")
    sinP = nc.dram_tensor("sinP", [HD, S], BF16, kind="ExternalInput")
    maskT = nc.dram_tensor("maskT", [128, 512], BF16, kind="ExternalInput")
    onesd = nc.dram_tensor("onesd", [128, 128], BF16, kind="ExternalInput")
    out = nc.dram_tensor("out", [S, D], BF16, kind="ExternalOutput")

    with TileContext(nc) as tc:
        with (
            tc.tile_pool(name="consts", bufs=1) as consts,
            tc.tile_pool(name="resid", bufs=1) as resid,
        ):
            # SBUF-resident q^T/k^T (per head, RoPE'd+permuted) and v, bf16
            q_sb = [resid.tile([HD, S], BF16, name=f"qT{h}") for h in range(HPC)]
            k_sb = [resid.tile([HD, S], BF16, name=f"kT{h}") for h in range(HPC)]
            v_sb = resid.tile([128, NKC, HL], BF16, name="v_sb")
            ctxs = [resid.tile([128, S], BF16, name=f"ctxT{h}") for h in range(HPC)]

            cos_sb = consts.tile([HD, S], BF16, name="cos_sb")
            sin_sb = consts.tile([HD, S], BF16, name="sin_sb")
            mask_sb = consts.tile([128, 512], BF16, name="mask_sb")
            ones_sb = consts.tile([128, 128], BF16, name="ones_sb")
            gpwarm = consts.tile([128, 128], F32, name="gpwarm")

            # ---------------- phase 1: projections + RoPE ----------------
            with (
                tc.tile_pool(name="wpool", bufs=1) as wpool,
                tc.tile_pool(name="xtp", bufs=2) as xtp,
                tc.tile_pool(name="st1", bufs=2) as st1,
                tc.tile_pool(name="psP", bufs=5, space="PSUM") as psP,
            ):
                w_sb = {}
                for nm in ("wq", "wk", "wv"):
                    w_sb[nm] = wpool.tile([128, NDC, HL], BF16, name=f"{nm}_sb")

                # ring priority: the first-needed bytes lead each queue
                # (only sync/scalar/gpsimd can issue DMAs, and gpsimd's
                # library load stalls its queue ~11us at boot).
                # sync: wv quarters, wq quarters, wk. scalar: x0 quarters,
                # RoPE tables, x1. gpsimd: warmup, then x2/x3.
                for qt in range(4):
                    nc.sync.dma_start(
                        out=w_sb["wv"][:, qt * 4:(qt + 1) * 4, :],
                        in_=wv[:, qt * 4:(qt + 1) * 4, :])

                def load_x_block(sb, quartered=False, eng=None):
                    eng = eng or nc.scalar
                    xt = xtp.tile([128, NDC, 512], BF16, tag="xt", name="xt")
                    if quartered:
                        for qt in range(4):
                            eng.dma_start(
                                out=xt[:, qt * 4:(qt + 1) * 4, :],
                                in_=xT[:, sb, qt * 4:(qt + 1) * 4, :])
                    else:
                        eng.dma_start(out=xt[:], in_=xT[:, sb])
                    return xt

                x_cur = load_x_block(0, quartered=True)
                for qt in range(4):
                    nc.sync.dma_start(
                        out=w_sb["wq"][:, qt * 4:(qt + 1) * 4, :],
                        in_=wq[:, qt * 4:(qt + 1) * 4, :])
                nc.scalar.dma_start(out=cos_sb[:], in_=cosP[:])
                nc.scalar.dma_start(out=sin_sb[:], in_=sinP[:])
                nc.scalar.dma_start(out=mask_sb[:], in_=maskT[:])
                nc.scalar.dma_start(out=ones_sb[:], in_=onesd[:])
                nc.sync.dma_start(out=w_sb["wk"][:], in_=wk[:])
                x_next = load_x_block(1)
                # warm the gpsimd tensor-op library off the critical path
                # (matches the phase-2 finalize multiply's op family)
                nc.gpsimd.tensor_mul(gpwarm[:], gpwarm[:], gpwarm[:])

                finishers = deque()

                def emit_finisher():
                    kind, args = finishers.popleft()
                    if kind == "qk":
                        ps, dst, sl = args
                        # rotate_half in permuted space = swap 64-halves; the
                        # swap copies run on ACT (idle in phase 1), the sign
                        # lives in sin_sb (host-folded)
                        qsw = st1.tile([128, 512], F32, tag="qsw", name="qsw")
                        nc.scalar.activation(qsw[0:64, :], ps[64:128, :],
                                             AF.Copy)
                        nc.scalar.activation(qsw[64:128, :], ps[0:64, :],
                                             AF.Copy)
                        ta = st1.tile([128, 512], F32, tag="ta", name="ta")
                        nc.vector.tensor_mul(ta[:], ps[:], cos_sb[:, sl])
                        tb = st1.tile([128, 512], F32, tag="tb", name="tb")
                        nc.vector.tensor_mul(tb[:], qsw[:], sin_sb[:, sl])
                        nc.vector.tensor_add(dst[:, sl], ta[:], tb[:])
                    else:
                        ps, kc = args
                        nc.scalar.activation(v_sb[:, kc, :], ps[:], AF.Copy)

                for sb in range(NSB):
                    sl = slice(sb * 512, (sb + 1) * 512)
                    if sb > 0:
                        x_cur = x_next
                        if sb < NSB - 1:
                            x_next = load_x_block(sb + 1, eng=nc.gpsimd)

                    # V first: no table dependency, so the RoPE tables get
                    # ~14us of slack on the vector ring
                    for sc in range(4):
                        ps = psP.tile([128, 512], F32, tag="p", name="ps")
                        for dc in range(NDC):
                            _mm(nc, ps[:],
                                x_cur[:, dc, sc * 128:(sc + 1) * 128],
                                w_sb["wv"][:, dc, :],
                                start=(dc == 0), stop=(dc == NDC - 1))
                        finishers.append(("v", (ps, sb * 4 + sc)))
                        if len(finishers) > FIN_LAG:
                            emit_finisher()

                    for wname, dst in (("wq", q_sb), ("wk", k_sb)):
                        w_t = w_sb[wname]
                        for h in range(HPC):
                            ps = psP.tile([128, 512], F32, tag="p", name="ps")
                            for dc in range(NDC):
                                _mm(nc, ps[:],
                                    w_t[:, dc, h * HD:(h + 1) * HD],
                                    x_cur[:, dc, :],
                                    start=(dc == 0), stop=(dc == NDC - 1))
                            finishers.append(("qk", (ps, dst[h], sl)))
                            if len(finishers) > FIN_LAG:
                                emit_finisher()
                while finishers:
                    emit_finisher()

            # ---------- phase 2+3: attention + output projection ----------
            with (
                tc.tile_pool(name="pp", bufs=7) as pp,
                tc.tile_pool(name="accp", bufs=3) as accp,
                tc.tile_pool(name="sm", bufs=2) as sm,
                tc.tile_pool(name="pvp", bufs=3) as pvp,
                tc.tile_pool(name="wop", bufs=1) as wop,
                tc.tile_pool(name="outp", bufs=4) as outp,
                tc.tile_pool(name="psA", bufs=3, space="PSUM") as psA,
                tc.tile_pool(name="psB", bufs=2, space="PSUM") as psB,
                tc.tile_pool(name="psC", bufs=1, space="PSUM") as psC,
                tc.tile_pool(name="psD", bufs=2, space="PSUM") as psD,
            ):
                wo_sb = wop.tile([128, HPC * D], BF16, name="wo_sb")
                nc.sync.dma_start(out=wo_sb[:], in_=wo[:])

                lagq = deque()
                Oq = deque()          # outproj sub-jobs (qc, db)
                done_cnt = [0] * NSB  # finalized heads per q-block

                def emit_lpv(job):
                    pv, pt, vtc, ncols, first, last, w = job
                    _mm(nc, pv[:, 512 - ncols:], vtc, pt[:, :ncols],
                        start=first, stop=last)
                    if last:
                        # eager PSUM evacuation (ACT) so the pv bank recycles
                        pvs = pvp.tile([128, 512], BF16, tag="pvs", name="pvs")
                        nc.scalar.activation(pvs[:], pv[:], AF.Copy)
                        w["pvs"] = pvs

                def emit_chain_a(w):
                    # replicated row-sum: ones[128,128]^T @ acc -> every
                    # partition holds l, so no partition_broadcast needed
                    lps = psC.tile([128, 512], F32, tag="c", name="lps")
                    _mm(nc, lps[:], ones_sb[:], w["acc"][:],
                        start=True, stop=True)
                    rcp = sm.tile([128, 512], F32, tag="rcp", name="rcp")
                    nc.vector.reciprocal_approx_fast(rcp[:], lps[:])
                    w["rcp"] = rcp

                def emit_chain_b(w):
                    qb = w["qb"]
                    # SBUF-only operands, so this can live on gpsimd
                    nc.gpsimd.tensor_mul(
                        ctxs[w["h"]][:, qb * 512:(qb + 1) * 512],
                        w["pvs"][:], w["rcp"][:])
                    done_cnt[qb] += 1
                    if done_cnt[qb] == HPC:
                        for qc in range(4 * qb, 4 * qb + 4):
                            for db in range(D // 512):
                                Oq.append((qc, db))

                def emit_outproj():
                    qc, db = Oq.popleft()
                    ops = psD.tile([128, 512], F32, tag="d", name="ops")
                    for hh in range(HPC):
                        _mm(nc, ops[:],
                            ctxs[hh][:, qc * 128:(qc + 1) * 128],
                            wo_sb[:, hh * D + db * 512:hh * D + (db + 1) * 512],
                            start=(hh == 0), stop=(hh == HPC - 1))
                    osb = outp.tile([128, 512], BF16, tag="osb", name="osb")
                    # gpsimd cannot read PSUM; alternate evacuation ACT/DVE
                    if (qc + db) % 2 == 0:
                        nc.scalar.activation(osb[:], ops[:], AF.Copy)
                    else:
                        nc.vector.tensor_copy(osb[:], ops[:])
                    nc.sync.dma_start(
                        out=out[qc * 128:(qc + 1) * 128,
                                db * 512:(db + 1) * 512],
                        in_=osb[:])

                prev_w = None
                for qb in range(NSB):
                    for h in range(HPC):
                        nk = 4 * qb + 4
                        w = {"h": h, "qb": qb}
                        pv = psB.tile([128, 512], F32, tag="b", name="pv")
                        acc = accp.tile([128, 512], BF16, tag="acc", name="acc")
                        w["acc"] = acc
                        for kc in range(nk):
                            j = kc - 4 * qb
                            ncols = 512 if j < 0 else 512 - 128 * j
                            sps = psA.tile([128, 512], F32, tag="a", name="sps")
                            _mm(nc, sps[:, :ncols],
                                k_sb[h][:, kc * 128:(kc + 1) * 128],
                                q_sb[h][:, qb * 512 + 512 - ncols:(qb + 1) * 512],
                                start=True, stop=True)
                            pt = pp.tile([128, 512], BF16, tag="pt", name="pt")
                            nc.scalar.activation(pt[:, :ncols], sps[:, :ncols],
                                                 AF.Exp)
                            # gpsimd is ~3x slower per elementwise op than
                            # DVE; keep the per-chunk critical path on DVE
                            if j >= 0:
                                nc.vector.tensor_mul(pt[:, :ncols],
                                                     pt[:, :ncols],
                                                     mask_sb[:, :ncols])
                            if kc == 0:
                                nc.vector.tensor_copy(acc[:], pt[:])
                            else:
                                nc.vector.tensor_add(acc[:, 512 - ncols:],
                                                     acc[:, 512 - ncols:],
                                                     pt[:, :ncols])
                            lagq.append((pv, pt,
                                         v_sb[:, kc, h * HD:(h + 1) * HD],
                                         ncols, kc == 0, kc == nk - 1, w))
                            while len(lagq) > PV_LAG:
                                emit_lpv(lagq.popleft())
                            # one side action per chunk; the chain lags one
                            # extra chunk so the PE's l-reduce never waits on
                            # the accumulation engines
                            if kc == 1 and prev_w is not None:
                                emit_chain_a(prev_w)
                            elif kc == 2 and prev_w is not None:
                                emit_chain_b(prev_w)
                            elif Oq:
                                emit_outproj()
                        prev_w = w
                while lagq:
                    emit_lpv(lagq.popleft())
                emit_chain_a(prev_w)
                emit_chain_b(prev_w)
                while Oq:
                    emit_outproj()

    nc.compile()
    return nc


_NC_CACHE = None


def _get_nc():
    global _NC_CACHE
    if _NC_CACHE is None:
        _NC_CACHE = _build()
    return _NC_CACHE


# de-interleave: evens then odds, per head
_PERM = np.concatenate([np.arange(0, HD, 2), np.arange(1, HD, 2)])


def _host_tables():
    # Replicate reference RoPE tables in float32 arithmetic, permuted.
    inv_freq = np.float32(1.0) / np.power(
        np.float32(ROPE_THETA), np.arange(0, HD, 2).astype(np.float32) / np.float32(HD)
    )
    pos = np.arange(S, dtype=np.float32)
    freqs = pos[:, None] * inv_freq[None, :]
    angles = np.concatenate([freqs, freqs], axis=1)  # [S, HD]
    cos = np.cos(angles).astype(np.float32)
    sin = np.sin(angles).astype(np.float32)
    cos_p = np.ascontiguousarray(cos[:, _PERM].T)  # [HD, S]
    sin_p = np.ascontiguousarray(sin[:, _PERM].T).copy()
    sin_p[:HD // 2] *= np.float32(-1.0)  # fold rotate_half's sign
    mask = (np.arange(128)[:, None] <= np.arange(512)[None, :]).astype(BNP)
    return cos_p.astype(BNP), sin_p.astype(BNP), mask


_ONES = np.ones((128, 128), dtype=BNP)


def kernel(x, Wq, Wk, Wv, Wo):
    x = np.asarray(x, dtype=np.float32)
    Wq = np.asarray(Wq, dtype=np.float32)
    Wk = np.asarray(Wk, dtype=np.float32)
    Wv = np.asarray(Wv, dtype=np.float32)
    Wo = np.asarray(Wo, dtype=np.float32)

    results = _run_device(x, Wq, Wk, Wv, Wo)

    out = np.empty((B, S, D), dtype=np.float32)
    for b in range(B):
        acc = results[b * CORES_PER_BATCH]["out"].astype(np.float32)
        for i in range(1, CORES_PER_BATCH):
            acc = acc + results[b * CORES_PER_BATCH + i]["out"].astype(np.float32)
        out[b] = acc
    return out


def _make_in_maps(x, Wq, Wk, Wv, Wo):
    cos_p, sin_p, mask = _host_tables()
    scale = np.float32(1.0 / math.sqrt(HD))
    # permutation of a 512-col (4-head) slice: de-interleave within each head
    block_perm = np.concatenate([hh * HD + _PERM for hh in range(HPC)])

    def dev_w(w):  # [D, HL_slice] -> [128, NDC, hl]
        return np.ascontiguousarray(
            w.reshape(NDC, 128, -1).transpose(1, 0, 2)).astype(BNP)

    wq_scaled = (Wq * scale).astype(np.float32)
    xTb = [
        np.ascontiguousarray(
            x[b].T.reshape(NDC, 128, NSB, 512).transpose(1, 2, 0, 3)).astype(BNP)
        for b in range(B)
    ]
    in_maps = []
    for c in range(N_CORES):
        b = c // CORES_PER_BATCH
        g = c % CORES_PER_BATCH
        hs = slice(g * HL, (g + 1) * HL)
        in_maps.append({
            "xT": xTb[b],
            "wq": dev_w(wq_scaled[:, hs][:, block_perm]),
            "wk": dev_w(Wk[:, hs][:, block_perm]),
            "wv": dev_w(Wv[:, hs]),
            "wo": np.ascontiguousarray(
                Wo[hs, :].reshape(HL // 128, 128, D).transpose(1, 0, 2)).astype(BNP),
            "cosP": cos_p,
            "sinP": sin_p,
            "maskT": mask,
            "onesd": _ONES,
        })
    return in_maps


def _run_device(x, Wq, Wk, Wv, Wo, trace=False):
    nc = _get_nc()
    in_maps = _make_in_maps(x, Wq, Wk, Wv, Wo)
    res = run_bass_kernel_spmd(nc, in_maps, core_ids=list(range(N_CORES)), trace=trace)
    if trace:
        return res
    return res.results


def run_traced(x, Wq, Wk, Wv, Wo):
    """Run with NTFF tracing; returns (full_output, BassKernelResults)."""
    res = _run_device(np.asarray(x, np.float32), np.asarray(Wq, np.float32),
                      np.asarray(Wk, np.float32), np.asarray(Wv, np.float32),
                      np.asarray(Wo, np.float32), trace=True)
    out = np.empty((B, S, D), dtype=np.float32)
    for b in range(B):
        acc = res.results[b * CORES_PER_BATCH]["out"].astype(np.float32)
        for i in range(1, CORES_PER_BATCH):
            acc = acc + res.results[b * CORES_PER_BATCH + i]["out"].astype(np.float32)
        out[b] = acc
    return out, res


# revision 21
# speedup vs baseline: 1.2352x; 1.0054x over previous
"""Causal self-attention (B=2, S=2048, D=2048, H=16, HD=128) on 8 TRN2 cores.

Sharding: core c -> batch b = c//4, heads 4*(c%4)..4*(c%4)+3 (tensor-parallel
over heads within a batch; data-parallel over batch across core groups).

v4 design (v3 ~400us measured):
  - RoPE via host-side de-interleave permutation of Wq/Wk columns (evens then
    odds per head) + permuted cos/sin tables with the rotate sign folded into
    sin. rotate_half becomes a partition-half swap, done by reading the
    projection PSUM at partition offset 64 in the DVE multiplies. Kills the
    pmat matmul and the qraw ACT evacuation of v3. Scores are invariant to the
    permutation (q,k both permuted); V path untouched.
  - head-priority DMA: V projections run first (no RoPE tables needed), with
    x0/wv quarters leading the scalar/sync rings and the (bf16) cos/sin
    tables on the vector ring. v3 put 2MB of f32 tables ahead of x0 which
    delayed the first matmul to 39us.
  - softmax row-sums fully off the PE: DVE/gpsimd accumulate exp chunks, one
    ones-matrix matmul per window reduces to a *replicated* [128,512] row-sum
    (M=128 costs the same as M=1), so the reciprocal needs no gpsimd
    partition_broadcast. reciprocal_approx_fast only (~18 bits, output is
    bf16 anyway).
  - qb-major window order: output-projection sub-jobs (4 matmuls each) become
    available 4 windows in and are dripped one per chunk, keeping the PE
    continuously fed (the PE clocks down after any idle and needs 3us of
    continuous execution to re-reach max p-state).
  - eager PV evacuation on ACT; outproj evacuation split DVE/gpsimd; mask
    multiplies + diagonal-chunk accumulation on gpsimd to keep every engine
    under ~70%.
"""

import math
from collections import deque

import ml_dtypes
import numpy as np

import concourse.bacc as bacc
import concourse.mybir as mybir
from concourse.tile import TileContext
from concourse.bass_utils import run_bass_kernel_spmd

B, S, D = 2, 2048, 2048
H, HD = 16, 128
ROPE_THETA = 10000.0

N_CORES = 8
CORES_PER_BATCH = 4
HPC = H // (N_CORES // B)  # heads per core = 4
HL = HPC * HD              # 512 local head-dim columns
NDC = D // 128             # 16 contraction chunks
NSB = S // 512             # 4 s-blocks
NKC = S // 128             # 16 k-chunks

F32 = mybir.dt.float32
BF16 = mybir.dt.bfloat16
AF = mybir.ActivationFunctionType
BNP = ml_dtypes.bfloat16

FIN_LAG = 3   # projection finishers lag this many groups
PV_LAG = 3    # PV matmuls lag score matmuls by this many chunks


def _mm(nc, out, lhsT, rhs, start, stop):
    nc.tensor.matmul(out, lhsT, rhs, start=start, stop=stop)


def _build():
    nc = bacc.Bacc("TRN2", target_bir_lowering=False, debug=False)

    # all big operands arrive pre-rearranged to the on-chip layout so each
    # DMA is one contiguous run per partition
    xT = nc.dram_tensor("xT", [128, NSB, NDC, 512], BF16, kind="ExternalInput")
    wq = nc.dram_tensor("wq", [128, NDC, HL], BF16, kind="ExternalInput")
    wk = nc.dram_tensor("wk", [128, NDC, HL], BF16, kind="ExternalInput")
    wv = nc.dram_tensor("wv", [128, NDC, HL], BF16, kind="ExternalInput")
    wo = nc.dram_tensor("wo", [128, HL // 128, D], BF16, kind="ExternalInput")
    cosP = nc.dram_tensor("cosP", [HD, S], BF16, kind="ExternalInput")
    sinP = nc.dram_tensor("sinP", [HD, S], BF16, kind="ExternalInput")
    maskT = nc.dram_tensor("maskT", [128, 512], BF16, kind="ExternalInput")
    onesd = nc.dram_tensor("onesd", [128, 128], BF16, kind="ExternalInput")
    out = nc.dram_tensor("out", [S, D], BF16, kind="ExternalOutput")

    with TileContext(nc) as tc:
        with (
            tc.tile_pool(name="consts", bufs=1) as consts,
            tc.tile_pool(name="resid", bufs=1) as resid,
        ):
            # SBUF-resident q^T/k^T (per head, RoPE'd+permuted) and v, bf16.
            # One tile per 512-block: tile-granular dependency tracking would
            # otherwise stall attention on block 0 until the LAST projection
            # finisher writes the shared tile (and create WAR hazards between
            # outproj reads and later finalize writes for ctx).
            q_sb = [[resid.tile([HD, 512], BF16, name=f"qT{h}_{b}")
                     for b in range(NSB)] for h in range(HPC)]
            k_sb = [[resid.tile([HD, 512], BF16, name=f"kT{h}_{b}")
                     for b in range(NSB)] for h in range(HPC)]
            v_sb = resid.tile([128, NKC, HL], BF16, name="v_sb")
            ctxs = [[resid.tile([128, 512], BF16, name=f"ctxT{h}_{b}")
                     for b in range(NSB)] for h in range(HPC)]

            cos_sb = consts.tile([HD, S], BF16, name="cos_sb")
            sin_sb = consts.tile([HD, S], BF16, name="sin_sb")
            mask_sb = consts.tile([128, 512], BF16, name="mask_sb")
            ones_sb = consts.tile([128, 128], BF16, name="ones_sb")
            gpwarm = consts.tile([128, 128], F32, name="gpwarm")

            # ---------------- phase 1: projections + RoPE ----------------
            with (
                tc.tile_pool(name="wpool", bufs=1) as wpool,
                tc.tile_pool(name="xtp", bufs=2) as xtp,
                tc.tile_pool(name="st1", bufs=2) as st1,
                tc.tile_pool(name="psP", bufs=5, space="PSUM") as psP,
            ):
                w_sb = {}
                for nm in ("wq", "wk", "wv"):
                    w_sb[nm] = wpool.tile([128, NDC, HL], BF16, name=f"{nm}_sb")

                # ring priority: the first-needed bytes lead each queue
                # (only sync/scalar/gpsimd can issue DMAs, and gpsimd's
                # library load stalls its queue ~11us at boot).
                # sync: wv quarters, wq quarters, wk. scalar: x0 quarters,
                # RoPE tables, x1. gpsimd: warmup, then x2/x3.
                for qt in range(8):
                    nc.sync.dma_start(
                        out=w_sb["wv"][:, qt * 2:(qt + 1) * 2, :],
                        in_=wv[:, qt * 2:(qt + 1) * 2, :])

                def load_x_block(sb, parts=1, eng=None):
                    eng = eng or nc.scalar
                    xt = xtp.tile([128, NDC, 512], BF16, tag="xt", name="xt")
                    step = NDC // parts
                    for qt in range(parts):
                        eng.dma_start(
                            out=xt[:, qt * step:(qt + 1) * step, :],
                            in_=xT[:, sb, qt * step:(qt + 1) * step, :])
                    return xt

                x_cur = load_x_block(0, parts=8)
                for qt in range(4):
                    nc.sync.dma_start(
                        out=w_sb["wq"][:, qt * 4:(qt + 1) * 4, :],
                        in_=wq[:, qt * 4:(qt + 1) * 4, :])
                nc.scalar.dma_start(out=cos_sb[:], in_=cosP[:])
                nc.scalar.dma_start(out=sin_sb[:], in_=sinP[:])
                nc.scalar.dma_start(out=mask_sb[:], in_=maskT[:])
                nc.scalar.dma_start(out=ones_sb[:], in_=onesd[:])
                nc.sync.dma_start(out=w_sb["wk"][:], in_=wk[:])
                x_next = load_x_block(1)
                # warm the gpsimd tensor-op library off the critical path
                # (matches the phase-2 finalize multiply's op family)
                nc.gpsimd.tensor_mul(gpwarm[:], gpwarm[:], gpwarm[:])

                finishers = deque()

                def emit_finisher():
                    kind, args = finishers.popleft()
                    if kind == "qk":
                        ps, dst, sb_i = args
                        sl = slice(sb_i * 512, (sb_i + 1) * 512)
                        # rotate_half in permuted space = swap 64-halves; the
                        # swap copies run on ACT (idle in phase 1), the sign
                        # lives in sin_sb (host-folded)
                        qsw = st1.tile([128, 512], F32, tag="qsw", name="qsw")
                        nc.scalar.activation(qsw[0:64, :], ps[64:128, :],
                                             AF.Copy)
                        nc.scalar.activation(qsw[64:128, :], ps[0:64, :],
                                             AF.Copy)
                        ta = st1.tile([128, 512], F32, tag="ta", name="ta")
                        nc.vector.tensor_mul(ta[:], ps[:], cos_sb[:, sl])
                        tb = st1.tile([128, 512], F32, tag="tb", name="tb")
                        nc.vector.tensor_mul(tb[:], qsw[:], sin_sb[:, sl])
                        nc.vector.tensor_add(dst[:], ta[:], tb[:])
                    else:
                        ps, kc = args
                        nc.scalar.activation(v_sb[:, kc, :], ps[:], AF.Copy)

                for sb in range(NSB):
                    if sb > 0:
                        x_cur = x_next
                        if sb < NSB - 1:
                            x_next = load_x_block(sb + 1, eng=nc.gpsimd)

                    # V first: no table dependency, so the RoPE tables get
                    # ~14us of slack on the scalar ring
                    for sc in range(4):
                        ps = psP.tile([128, 512], F32, tag="p", name="ps")
                        for dc in range(NDC):
                            _mm(nc, ps[:],
                                x_cur[:, dc, sc * 128:(sc + 1) * 128],
                                w_sb["wv"][:, dc, :],
                                start=(dc == 0), stop=(dc == NDC - 1))
                        finishers.append(("v", (ps, sb * 4 + sc)))
                        if len(finishers) > FIN_LAG:
                            emit_finisher()

                    for wname, dst in (("wq", q_sb), ("wk", k_sb)):
                        w_t = w_sb[wname]
                        for h in range(HPC):
                            ps = psP.tile([128, 512], F32, tag="p", name="ps")
                            for dc in range(NDC):
                                _mm(nc, ps[:],
                                    w_t[:, dc, h * HD:(h + 1) * HD],
                                    x_cur[:, dc, :],
                                    start=(dc == 0), stop=(dc == NDC - 1))
                            finishers.append(("qk", (ps, dst[h][sb], sb)))
                            if len(finishers) > FIN_LAG:
                                emit_finisher()
                while finishers:
                    emit_finisher()

            # ---------- phase 2+3: attention + output projection ----------
            with (
                tc.tile_pool(name="pp", bufs=7) as pp,
                tc.tile_pool(name="accp", bufs=3) as accp,
                tc.tile_pool(name="sm", bufs=2) as sm,
                tc.tile_pool(name="pvp", bufs=3) as pvp,
                tc.tile_pool(name="wop", bufs=1) as wop,
                tc.tile_pool(name="outp", bufs=4) as outp,
                tc.tile_pool(name="psA", bufs=3, space="PSUM") as psA,
                tc.tile_pool(name="psB", bufs=2, space="PSUM") as psB,
                tc.tile_pool(name="psC", bufs=1, space="PSUM") as psC,
                tc.tile_pool(name="psD", bufs=2, space="PSUM") as psD,
            ):
                wo_sb = wop.tile([128, HPC * D], BF16, name="wo_sb")
                nc.sync.dma_start(out=wo_sb[:], in_=wo[:])

                lagq = deque()
                Oq = deque()          # outproj sub-jobs (qc, db)
                done_cnt = [0] * NSB  # finalized heads per q-block

                def emit_lpv(job):
                    pv, pt, vtc, ncols, first, last, w = job
                    _mm(nc, pv[:, 512 - ncols:], vtc, pt[:, :ncols],
                        start=first, stop=last)
                    if last:
                        # eager PSUM evacuation (ACT) so the pv bank recycles
                        pvs = pvp.tile([128, 512], BF16, tag="pvs", name="pvs")
                        nc.scalar.activation(pvs[:], pv[:], AF.Copy)
                        w["pvs"] = pvs

                def emit_chain_a(w):
                    # replicated row-sum: ones[128,128]^T @ acc -> every
                    # partition holds l, so no partition_broadcast needed
                    lps = psC.tile([128, 512], F32, tag="c", name="lps")
                    _mm(nc, lps[:], ones_sb[:], w["acc"][:],
                        start=True, stop=True)
                    rcp = sm.tile([128, 512], F32, tag="rcp", name="rcp")
                    nc.vector.reciprocal_approx_fast(rcp[:], lps[:])
                    w["rcp"] = rcp

                def emit_chain_b(w, eng=None):
                    qb = w["qb"]
                    # SBUF-only operands, so this can live on gpsimd
                    (eng or nc.gpsimd).tensor_mul(
                        ctxs[w["h"]][qb][:], w["pvs"][:], w["rcp"][:])
                    done_cnt[qb] += 1
                    if done_cnt[qb] == HPC:
                        for qc in range(4 * qb, 4 * qb + 4):
                            for db in range(D // 512):
                                Oq.append((qc, db))

                def emit_outproj(pool=None, tag="d"):
                    qc, db = Oq.popleft()
                    ops = (pool or psD).tile([128, 512], F32, tag=tag,
                                             name="ops")
                    for hh in range(HPC):
                        _mm(nc, ops[:],
                            ctxs[hh][qc // 4][:, (qc % 4) * 128:
                                              (qc % 4 + 1) * 128],
                            wo_sb[:, hh * D + db * 512:hh * D + (db + 1) * 512],
                            start=(hh == 0), stop=(hh == HPC - 1))
                    osb = outp.tile([128, 512], BF16, tag="osb", name="osb")
                    # gpsimd cannot read PSUM; alternate evacuation ACT/DVE
                    if (qc + db) % 2 == 0:
                        nc.scalar.activation(osb[:], ops[:], AF.Copy)
                    else:
                        nc.vector.tensor_copy(osb[:], ops[:])
                    nc.sync.dma_start(
                        out=out[qc * 128:(qc + 1) * 128,
                                db * 512:(db + 1) * 512],
                        in_=osb[:])

                prev_w = None
                for qb in range(NSB):
                    for h in range(HPC):
                        nk = 4 * qb + 4
                        w = {"h": h, "qb": qb}
                        pv = psB.tile([128, 512], F32, tag="b", name="pv")
                        acc = accp.tile([128, 512], BF16, tag="acc", name="acc")
                        w["acc"] = acc
                        # diagonal (masked) chunks first: their 2 DVE ops per
                        # chunk then overlap the previous window's light tail
                        # instead of piling up right before the next window's
                        # l-reduce
                        order = list(range(4 * qb, nk)) + list(range(0, 4 * qb))
                        for ci, kc in enumerate(order):
                            j = kc - 4 * qb
                            ncols = 512 if j < 0 else 512 - 128 * j
                            sps = psA.tile([128, 512], F32, tag="a", name="sps")
                            _mm(nc, sps[:, :ncols],
                                k_sb[h][kc // 4][:, (kc % 4) * 128:
                                                 (kc % 4 + 1) * 128],
                                q_sb[h][qb][:, 512 - ncols:],
                                start=True, stop=True)
                            pt = pp.tile([128, 512], BF16, tag="pt", name="pt")
                            nc.scalar.activation(pt[:, :ncols], sps[:, :ncols],
                                                 AF.Exp)
                            # gpsimd is ~3x slower per elementwise op than
                            # DVE; keep the per-chunk critical path on DVE
                            if j >= 0:
                                nc.vector.tensor_mul(pt[:, :ncols],
                                                     pt[:, :ncols],
                                                     mask_sb[:, :ncols])
                            if ci == 0:
                                nc.vector.tensor_copy(acc[:], pt[:])
                            else:
                                nc.vector.tensor_add(acc[:, 512 - ncols:],
                                                     acc[:, 512 - ncols:],
                                                     pt[:, :ncols])
                            lagq.append((pv, pt,
                                         v_sb[:, kc, h * HD:(h + 1) * HD],
                                         ncols, ci == 0, ci == nk - 1, w))
                            while len(lagq) > PV_LAG:
                                emit_lpv(lagq.popleft())
                            # one side action per chunk; the chain lags a few
                            # chunks so the PE's l-reduce never waits on the
                            # accumulation engines
                            if ci == 2 and prev_w is not None:
                                emit_chain_a(prev_w)
                            elif ci == 3 and prev_w is not None:
                                emit_chain_b(prev_w)
                            elif Oq:
                                emit_outproj()
                        prev_w = w
                while lagq:
                    emit_lpv(lagq.popleft())
                emit_chain_a(prev_w)
                emit_chain_b(prev_w, eng=nc.vector)
                # final outproj drain: alternate two PSUM pools so the next
                # job's matmuls never wait on the previous evacuation
                flip = 0
                while Oq:
                    if flip % 2 == 0:
                        emit_outproj(pool=psD, tag="d")
                    else:
                        emit_outproj(pool=psA, tag="a")
                    flip += 1

    nc.compile()
    return nc


_NC_CACHE = None


def _get_nc():
    global _NC_CACHE
    if _NC_CACHE is None:
        _NC_CACHE = _build()
    return _NC_CACHE


# de-interleave: evens then odds, per head
_PERM = np.concatenate([np.arange(0, HD, 2), np.arange(1, HD, 2)])


def _host_tables():
    # Replicate reference RoPE tables in float32 arithmetic, permuted.
    inv_freq = np.float32(1.0) / np.power(
        np.float32(ROPE_THETA), np.arange(0, HD, 2).astype(np.float32) / np.float32(HD)
    )
    pos = np.arange(S, dtype=np.float32)
    freqs = pos[:, None] * inv_freq[None, :]
    angles = np.concatenate([freqs, freqs], axis=1)  # [S, HD]
    cos = np.cos(angles).astype(np.float32)
    sin = np.sin(angles).astype(np.float32)
    cos_p = np.ascontiguousarray(cos[:, _PERM].T)  # [HD, S]
    sin_p = np.ascontiguousarray(sin[:, _PERM].T).copy()
    sin_p[:HD // 2] *= np.float32(-1.0)  # fold rotate_half's sign
    mask = (np.arange(128)[:, None] <= np.arange(512)[None, :]).astype(BNP)
    return cos_p.astype(BNP), sin_p.astype(BNP), mask


_ONES = np.ones((128, 128), dtype=BNP)


def kernel(x, Wq, Wk, Wv, Wo):
    x = np.asarray(x, dtype=np.float32)
    Wq = np.asarray(Wq, dtype=np.float32)
    Wk = np.asarray(Wk, dtype=np.float32)
    Wv = np.asarray(Wv, dtype=np.float32)
    Wo = np.asarray(Wo, dtype=np.float32)

    results = _run_device(x, Wq, Wk, Wv, Wo)

    out = np.empty((B, S, D), dtype=np.float32)
    for b in range(B):
        acc = results[b * CORES_PER_BATCH]["out"].astype(np.float32)
        for i in range(1, CORES_PER_BATCH):
            acc = acc + results[b * CORES_PER_BATCH + i]["out"].astype(np.float32)
        out[b] = acc
    return out


def _make_in_maps(x, Wq, Wk, Wv, Wo):
    cos_p, sin_p, mask = _host_tables()
    scale = np.float32(1.0 / math.sqrt(HD))
    # permutation of a 512-col (4-head) slice: de-interleave within each head
    block_perm = np.concatenate([hh * HD + _PERM for hh in range(HPC)])

    def dev_w(w):  # [D, HL_slice] -> [128, NDC, hl]
        return np.ascontiguousarray(
            w.reshape(NDC, 128, -1).transpose(1, 0, 2)).astype(BNP)

    wq_scaled = (Wq * scale).astype(np.float32)
    xTb = [
        np.ascontiguousarray(
            x[b].T.reshape(NDC, 128, NSB, 512).transpose(1, 2, 0, 3)).astype(BNP)
        for b in range(B)
    ]
    in_maps = []
    for c in range(N_CORES):
        b = c // CORES_PER_BATCH
        g = c % CORES_PER_BATCH
        hs = slice(g * HL, (g + 1) * HL)
        in_maps.append({
            "xT": xTb[b],
            "wq": dev_w(wq_scaled[:, hs][:, block_perm]),
            "wk": dev_w(Wk[:, hs][:, block_perm]),
            "wv": dev_w(Wv[:, hs]),
            "wo": np.ascontiguousarray(
                Wo[hs, :].reshape(HL // 128, 128, D).transpose(1, 0, 2)).astype(BNP),
            "cosP": cos_p,
            "sinP": sin_p,
            "maskT": mask,
            "onesd": _ONES,
        })
    return in_maps


def _run_device(x, Wq, Wk, Wv, Wo, trace=False):
    nc = _get_nc()
    in_maps = _make_in_maps(x, Wq, Wk, Wv, Wo)
    res = run_bass_kernel_spmd(nc, in_maps, core_ids=list(range(N_CORES)), trace=trace)
    if trace:
        return res
    return res.results


def run_traced(x, Wq, Wk, Wv, Wo):
    """Run with NTFF tracing; returns (full_output, BassKernelResults)."""
    res = _run_device(np.asarray(x, np.float32), np.asarray(Wq, np.float32),
                      np.asarray(Wk, np.float32), np.asarray(Wv, np.float32),
                      np.asarray(Wo, np.float32), trace=True)
    out = np.empty((B, S, D), dtype=np.float32)
    for b in range(B):
        acc = res.results[b * CORES_PER_BATCH]["out"].astype(np.float32)
        for i in range(1, CORES_PER_BATCH):
            acc = acc + res.results[b * CORES_PER_BATCH + i]["out"].astype(np.float32)
        out[b] = acc
    return out, res


# revision 26
# speedup vs baseline: 1.2358x; 1.0005x over previous
"""Causal self-attention (B=2, S=2048, D=2048, H=16, HD=128) on 8 TRN2 cores.

Sharding: core c -> batch b = c//4, heads 4*(c%4)..4*(c%4)+3 (tensor-parallel
over heads within a batch; data-parallel over batch across core groups).

v4 design (v3 ~400us measured):
  - RoPE via host-side de-interleave permutation of Wq/Wk columns (evens then
    odds per head) + permuted cos/sin tables with the rotate sign folded into
    sin. rotate_half becomes a partition-half swap, done by reading the
    projection PSUM at partition offset 64 in the DVE multiplies. Kills the
    pmat matmul and the qraw ACT evacuation of v3. Scores are invariant to the
    permutation (q,k both permuted); V path untouched.
  - head-priority DMA: V projections run first (no RoPE tables needed), with
    x0/wv quarters leading the scalar/sync rings and the (bf16) cos/sin
    tables on the vector ring. v3 put 2MB of f32 tables ahead of x0 which
    delayed the first matmul to 39us.
  - softmax row-sums fully off the PE: DVE/gpsimd accumulate exp chunks, one
    ones-matrix matmul per window reduces to a *replicated* [128,512] row-sum
    (M=128 costs the same as M=1), so the reciprocal needs no gpsimd
    partition_broadcast. reciprocal_approx_fast only (~18 bits, output is
    bf16 anyway).
  - qb-major window order: output-projection sub-jobs (4 matmuls each) become
    available 4 windows in and are dripped one per chunk, keeping the PE
    continuously fed (the PE clocks down after any idle and needs 3us of
    continuous execution to re-reach max p-state).
  - eager PV evacuation on ACT; outproj evacuation split DVE/gpsimd; mask
    multiplies + diagonal-chunk accumulation on gpsimd to keep every engine
    under ~70%.
"""

import math
from collections import deque

import ml_dtypes
import numpy as np

import concourse.bacc as bacc
import concourse.mybir as mybir
from concourse.tile import TileContext
from concourse.bass_utils import run_bass_kernel_spmd

B, S, D = 2, 2048, 2048
H, HD = 16, 128
ROPE_THETA = 10000.0

N_CORES = 8
CORES_PER_BATCH = 4
HPC = H // (N_CORES // B)  # heads per core = 4
HL = HPC * HD              # 512 local head-dim columns
NDC = D // 128             # 16 contraction chunks
NSB = S // 512             # 4 s-blocks
NKC = S // 128             # 16 k-chunks

F32 = mybir.dt.float32
BF16 = mybir.dt.bfloat16
AF = mybir.ActivationFunctionType
BNP = ml_dtypes.bfloat16

FIN_LAG = 2   # projection finishers lag this many groups
PV_LAG = 3    # PV matmuls lag score matmuls by this many chunks


def _mm(nc, out, lhsT, rhs, start, stop):
    nc.tensor.matmul(out, lhsT, rhs, start=start, stop=stop)


def _build():
    nc = bacc.Bacc("TRN2", target_bir_lowering=False, debug=False)

    # all big operands arrive pre-rearranged to the on-chip layout so each
    # DMA is one contiguous run per partition
    xT = nc.dram_tensor("xT", [128, NSB, NDC, 512], BF16, kind="ExternalInput")
    wq = nc.dram_tensor("wq", [128, NDC, HL], BF16, kind="ExternalInput")
    wk = nc.dram_tensor("wk", [128, NDC, HL], BF16, kind="ExternalInput")
    wv = nc.dram_tensor("wv", [128, NDC, HL], BF16, kind="ExternalInput")
    wo = nc.dram_tensor("wo", [128, HL // 128, D], BF16, kind="ExternalInput")
    cosP = nc.dram_tensor("cosP", [HD, S], BF16, kind="ExternalInput")
    sinP = nc.dram_tensor("sinP", [HD, S], BF16, kind="ExternalInput")
    maskT = nc.dram_tensor("maskT", [128, 512], BF16, kind="ExternalInput")
    onesd = nc.dram_tensor("onesd", [128, 128], BF16, kind="ExternalInput")
    out = nc.dram_tensor("out", [S, D], BF16, kind="ExternalOutput")

    with TileContext(nc) as tc:
        with (
            tc.tile_pool(name="consts", bufs=1) as consts,
            tc.tile_pool(name="resid", bufs=1) as resid,
            # PSUM pools are shared across both phases: closing/reopening
            # them would make the first attention matmul wait on the LAST
            # projection finisher's PSUM read (bank-reuse hazard)
            tc.tile_pool(name="psA", bufs=3, space="PSUM") as psA,
            tc.tile_pool(name="psB", bufs=2, space="PSUM") as psB,
            tc.tile_pool(name="psC", bufs=1, space="PSUM") as psC,
            tc.tile_pool(name="psD", bufs=2, space="PSUM") as psD,
        ):
            # SBUF-resident q^T/k^T (per head, RoPE'd+permuted) and v, bf16.
            # One tile per 512-block: tile-granular dependency tracking would
            # otherwise stall attention on block 0 until the LAST projection
            # finisher writes the shared tile (and create WAR hazards between
            # outproj reads and later finalize writes for ctx).
            q_sb = [[resid.tile([HD, 512], BF16, name=f"qT{h}_{b}")
                     for b in range(NSB)] for h in range(HPC)]
            k_sb = [[resid.tile([HD, 512], BF16, name=f"kT{h}_{b}")
                     for b in range(NSB)] for h in range(HPC)]
            v_sb = resid.tile([128, NKC, HL], BF16, name="v_sb")
            ctxs = [[resid.tile([128, 512], BF16, name=f"ctxT{h}_{b}")
                     for b in range(NSB)] for h in range(HPC)]

            cos_sb = consts.tile([HD, S], BF16, name="cos_sb")
            sin_sb = consts.tile([HD, S], BF16, name="sin_sb")
            mask_sb = consts.tile([128, 512], BF16, name="mask_sb")
            ones_sb = consts.tile([128, 128], BF16, name="ones_sb")
            gpwarm = consts.tile([128, 128], F32, name="gpwarm")

            # ---------------- phase 1: projections + RoPE ----------------
            with (
                tc.tile_pool(name="wpool", bufs=1) as wpool,
                tc.tile_pool(name="xtp", bufs=2) as xtp,
                tc.tile_pool(name="st1", bufs=2) as st1,
            ):
                w_sb = {}
                for nm in ("wq", "wk", "wv"):
                    w_sb[nm] = wpool.tile([128, NDC, HL], BF16, name=f"{nm}_sb")

                # ring priority: the first-needed bytes lead each queue
                # (only sync/scalar/gpsimd can issue DMAs, and gpsimd's
                # library load stalls its queue ~11us at boot).
                # sync: wv quarters, wq quarters, wk. scalar: x0 quarters,
                # RoPE tables, x1. gpsimd: warmup, then x2/x3.
                for qt in range(8):
                    nc.sync.dma_start(
                        out=w_sb["wv"][:, qt * 2:(qt + 1) * 2, :],
                        in_=wv[:, qt * 2:(qt + 1) * 2, :])

                def load_x_block(sb, parts=1, eng=None):
                    eng = eng or nc.scalar
                    xt = xtp.tile([128, NDC, 512], BF16, tag="xt", name="xt")
                    step = NDC // parts
                    for qt in range(parts):
                        eng.dma_start(
                            out=xt[:, qt * step:(qt + 1) * step, :],
                            in_=xT[:, sb, qt * step:(qt + 1) * step, :])
                    return xt

                x_cur = load_x_block(0, parts=8)
                for qt in range(4):
                    nc.sync.dma_start(
                        out=w_sb["wq"][:, qt * 4:(qt + 1) * 4, :],
                        in_=wq[:, qt * 4:(qt + 1) * 4, :])
                nc.scalar.dma_start(out=cos_sb[:], in_=cosP[:])
                nc.scalar.dma_start(out=sin_sb[:], in_=sinP[:])
                nc.scalar.dma_start(out=mask_sb[:], in_=maskT[:])
                nc.scalar.dma_start(out=ones_sb[:], in_=onesd[:])
                nc.sync.dma_start(out=w_sb["wk"][:], in_=wk[:])
                x_next = load_x_block(1)
                # warm the gpsimd tensor-op library off the critical path
                # (matches the phase-2 finalize multiply's op family)
                nc.gpsimd.tensor_mul(gpwarm[:], gpwarm[:], gpwarm[:])

                finishers = deque()

                def emit_finisher():
                    kind, args = finishers.popleft()
                    if kind == "qk":
                        ps, dst, sb_i = args
                        sl = slice(sb_i * 512, (sb_i + 1) * 512)
                        # rotate_half in permuted space = swap 64-halves; the
                        # swap copies run on ACT (idle in phase 1), the sign
                        # lives in sin_sb (host-folded)
                        qsw = st1.tile([128, 512], F32, tag="qsw", name="qsw")
                        nc.scalar.activation(qsw[0:64, :], ps[64:128, :],
                                             AF.Copy)
                        nc.scalar.activation(qsw[64:128, :], ps[0:64, :],
                                             AF.Copy)
                        ta = st1.tile([128, 512], F32, tag="ta", name="ta")
                        nc.vector.tensor_mul(ta[:], ps[:], cos_sb[:, sl])
                        tb = st1.tile([128, 512], F32, tag="tb", name="tb")
                        nc.vector.tensor_mul(tb[:], qsw[:], sin_sb[:, sl])
                        nc.vector.tensor_add(dst[:], ta[:], tb[:])
                    else:
                        ps, kc = args
                        nc.scalar.activation(v_sb[:, kc, :], ps[:], AF.Copy)

                for sb in range(NSB):
                    if sb > 0:
                        x_cur = x_next
                        if sb < NSB - 1:
                            x_next = load_x_block(sb + 1, eng=nc.gpsimd)

                    # V first: no table dependency, so the RoPE tables get
                    # ~14us of slack on the scalar ring
                    for sc in range(4):
                        ps = psA.tile([128, 512], F32, tag="a", name="ps")
                        for dc in range(NDC):
                            _mm(nc, ps[:],
                                x_cur[:, dc, sc * 128:(sc + 1) * 128],
                                w_sb["wv"][:, dc, :],
                                start=(dc == 0), stop=(dc == NDC - 1))
                        finishers.append(("v", (ps, sb * 4 + sc)))
                        if len(finishers) > FIN_LAG:
                            emit_finisher()

                    for wname, dst in (("wq", q_sb), ("wk", k_sb)):
                        w_t = w_sb[wname]
                        for h in range(HPC):
                            ps = psA.tile([128, 512], F32, tag="a", name="ps")
                            for dc in range(NDC):
                                _mm(nc, ps[:],
                                    w_t[:, dc, h * HD:(h + 1) * HD],
                                    x_cur[:, dc, :],
                                    start=(dc == 0), stop=(dc == NDC - 1))
                            finishers.append(("qk", (ps, dst[h][sb], sb)))
                            if len(finishers) > FIN_LAG:
                                emit_finisher()
                while finishers:
                    emit_finisher()

            # ---------- phase 2+3: attention + output projection ----------
            with (
                tc.tile_pool(name="pp", bufs=7) as pp,
                tc.tile_pool(name="accp", bufs=3) as accp,
                tc.tile_pool(name="sm", bufs=2) as sm,
                tc.tile_pool(name="pvp", bufs=3) as pvp,
                tc.tile_pool(name="wop", bufs=1) as wop,
                tc.tile_pool(name="outp", bufs=4) as outp,
            ):
                wo_sb = wop.tile([128, HPC * D], BF16, name="wo_sb")
                nc.sync.dma_start(out=wo_sb[:], in_=wo[:])

                lagq = deque()
                Oq = deque()          # outproj sub-jobs (qc, db)
                done_cnt = [0] * NSB  # finalized heads per q-block

                def emit_lpv(job):
                    pv, pt, vtc, ncols, first, last, w = job
                    _mm(nc, pv[:, 512 - ncols:], vtc, pt[:, :ncols],
                        start=first, stop=last)
                    if last:
                        # eager PSUM evacuation (ACT) so the pv bank recycles
                        pvs = pvp.tile([128, 512], BF16, tag="pvs", name="pvs")
                        nc.scalar.activation(pvs[:], pv[:], AF.Copy)
                        w["pvs"] = pvs

                def emit_chain_a(w):
                    # replicated row-sum: ones[128,128]^T @ acc -> every
                    # partition holds l, so no partition_broadcast needed
                    lps = psC.tile([128, 512], F32, tag="c", name="lps")
                    _mm(nc, lps[:], ones_sb[:], w["acc"][:],
                        start=True, stop=True)
                    rcp = sm.tile([128, 512], F32, tag="rcp", name="rcp")
                    nc.vector.reciprocal_approx_fast(rcp[:], lps[:])
                    w["rcp"] = rcp

                def emit_chain_b(w, eng=None):
                    qb = w["qb"]
                    # SBUF-only operands, so this can live on gpsimd
                    (eng or nc.gpsimd).tensor_mul(
                        ctxs[w["h"]][qb][:], w["pvs"][:], w["rcp"][:])
                    done_cnt[qb] += 1
                    if done_cnt[qb] == HPC:
                        for qc in range(4 * qb, 4 * qb + 4):
                            for db in range(D // 512):
                                Oq.append((qc, db))

                def emit_outproj(pool=None, tag="d"):
                    qc, db = Oq.popleft()
                    ops = (pool or psD).tile([128, 512], F32, tag=tag,
                                             name="ops")
                    for hh in range(HPC):
                        _mm(nc, ops[:],
                            ctxs[hh][qc // 4][:, (qc % 4) * 128:
                                              (qc % 4 + 1) * 128],
                            wo_sb[:, hh * D + db * 512:hh * D + (db + 1) * 512],
                            start=(hh == 0), stop=(hh == HPC - 1))
                    osb = outp.tile([128, 512], BF16, tag="osb", name="osb")
                    # gpsimd cannot read PSUM; alternate evacuation ACT/DVE
                    if (qc + db) % 2 == 0:
                        nc.scalar.activation(osb[:], ops[:], AF.Copy)
                    else:
                        nc.vector.tensor_copy(osb[:], ops[:])
                    nc.sync.dma_start(
                        out=out[qc * 128:(qc + 1) * 128,
                                db * 512:(db + 1) * 512],
                        in_=osb[:])

                prev_w = None
                for qb in range(NSB):
                    for h in range(HPC):
                        nk = 4 * qb + 4
                        w = {"h": h, "qb": qb}
                        pv = psB.tile([128, 512], F32, tag="b", name="pv")
                        acc = accp.tile([128, 512], BF16, tag="acc", name="acc")
                        w["acc"] = acc
                        # diagonal (masked) chunks first: their 2 DVE ops per
                        # chunk then overlap the previous window's light tail
                        # instead of piling up right before the next window's
                        # l-reduce
                        order = list(range(4 * qb, nk)) + list(range(0, 4 * qb))
                        for ci, kc in enumerate(order):
                            j = kc - 4 * qb
                            ncols = 512 if j < 0 else 512 - 128 * j
                            sps = psA.tile([128, 512], F32, tag="a", name="sps")
                            _mm(nc, sps[:, :ncols],
                                k_sb[h][kc // 4][:, (kc % 4) * 128:
                                                 (kc % 4 + 1) * 128],
                                q_sb[h][qb][:, 512 - ncols:],
                                start=True, stop=True)
                            pt = pp.tile([128, 512], BF16, tag="pt", name="pt")
                            nc.scalar.activation(pt[:, :ncols], sps[:, :ncols],
                                                 AF.Exp)
                            # gpsimd is ~3x slower per elementwise op than
                            # DVE; keep the per-chunk critical path on DVE
                            if j >= 0:
                                nc.vector.tensor_mul(pt[:, :ncols],
                                                     pt[:, :ncols],
                                                     mask_sb[:, :ncols])
                            if ci == 0:
                                nc.vector.tensor_copy(acc[:], pt[:])
                            else:
                                nc.vector.tensor_add(acc[:, 512 - ncols:],
                                                     acc[:, 512 - ncols:],
                                                     pt[:, :ncols])
                            lagq.append((pv, pt,
                                         v_sb[:, kc, h * HD:(h + 1) * HD],
                                         ncols, ci == 0, ci == nk - 1, w))
                            while len(lagq) > PV_LAG:
                                emit_lpv(lagq.popleft())
                            # one side action per chunk. Window starts are
                            # ACT-exp-limited (small diag matmuls rip ahead of
                            # the exp chain), so ci 0/1 carry outproj fill to
                            # keep the PE fed; the l-chain runs at ci 2/3 so
                            # it never waits on the accumulation engines.
                            if ci < 2:
                                if Oq:
                                    emit_outproj()
                            elif ci == 2 and prev_w is not None:
                                emit_chain_a(prev_w)
                            elif ci == 3 and prev_w is not None:
                                emit_chain_b(prev_w)
                            elif Oq:
                                emit_outproj()
                        prev_w = w
                while lagq:
                    emit_lpv(lagq.popleft())
                emit_chain_a(prev_w)
                emit_chain_b(prev_w, eng=nc.vector)
                # final outproj drain: alternate two PSUM pools so the next
                # job's matmuls never wait on the previous evacuation
                flip = 0
                while Oq:
                    if flip % 2 == 0:
                        emit_outproj(pool=psD, tag="d")
                    else:
                        emit_outproj(pool=psA, tag="a")
                    flip += 1

    nc.compile()
    return nc


_NC_CACHE = None


def _get_nc():
    global _NC_CACHE
    if _NC_CACHE is None:
        _NC_CACHE = _build()
    return _NC_CACHE


# de-interleave: evens then odds, per head
_PERM = np.concatenate([np.arange(0, HD, 2), np.arange(1, HD, 2)])


def _host_tables():
    # Replicate reference RoPE tables in float32 arithmetic, permuted.
    inv_freq = np.float32(1.0) / np.power(
        np.float32(ROPE_THETA), np.arange(0, HD, 2).astype(np.float32) / np.float32(HD)
    )
    pos = np.arange(S, dtype=np.float32)
    freqs = pos[:, None] * inv_freq[None, :]
    angles = np.concatenate([freqs, freqs], axis=1)  # [S, HD]
    cos = np.cos(angles).astype(np.float32)
    sin = np.sin(angles).astype(np.float32)
    cos_p = np.ascontiguousarray(cos[:, _PERM].T)  # [HD, S]
    sin_p = np.ascontiguousarray(sin[:, _PERM].T).copy()
    sin_p[:HD // 2] *= np.float32(-1.0)  # fold rotate_half's sign
    mask = (np.arange(128)[:, None] <= np.arange(512)[None, :]).astype(BNP)
    return cos_p.astype(BNP), sin_p.astype(BNP), mask


_ONES = np.ones((128, 128), dtype=BNP)


def kernel(x, Wq, Wk, Wv, Wo):
    x = np.asarray(x, dtype=np.float32)
    Wq = np.asarray(Wq, dtype=np.float32)
    Wk = np.asarray(Wk, dtype=np.float32)
    Wv = np.asarray(Wv, dtype=np.float32)
    Wo = np.asarray(Wo, dtype=np.float32)

    results = _run_device(x, Wq, Wk, Wv, Wo)

    out = np.empty((B, S, D), dtype=np.float32)
    for b in range(B):
        acc = results[b * CORES_PER_BATCH]["out"].astype(np.float32)
        for i in range(1, CORES_PER_BATCH):
            acc = acc + results[b * CORES_PER_BATCH + i]["out"].astype(np.float32)
        out[b] = acc
    return out


def _make_in_maps(x, Wq, Wk, Wv, Wo):
    cos_p, sin_p, mask = _host_tables()
    scale = np.float32(1.0 / math.sqrt(HD))
    # permutation of a 512-col (4-head) slice: de-interleave within each head
    block_perm = np.concatenate([hh * HD + _PERM for hh in range(HPC)])

    def dev_w(w):  # [D, HL_slice] -> [128, NDC, hl]
        return np.ascontiguousarray(
            w.reshape(NDC, 128, -1).transpose(1, 0, 2)).astype(BNP)

    wq_scaled = (Wq * scale).astype(np.float32)
    xTb = [
        np.ascontiguousarray(
            x[b].T.reshape(NDC, 128, NSB, 512).transpose(1, 2, 0, 3)).astype(BNP)
        for b in range(B)
    ]
    in_maps = []
    for c in range(N_CORES):
        b = c // CORES_PER_BATCH
        g = c % CORES_PER_BATCH
        hs = slice(g * HL, (g + 1) * HL)
        in_maps.append({
            "xT": xTb[b],
            "wq": dev_w(wq_scaled[:, hs][:, block_perm]),
            "wk": dev_w(Wk[:, hs][:, block_perm]),
            "wv": dev_w(Wv[:, hs]),
            "wo": np.ascontiguousarray(
                Wo[hs, :].reshape(HL // 128, 128, D).transpose(1, 0, 2)).astype(BNP),
            "cosP": cos_p,
            "sinP": sin_p,
            "maskT": mask,
            "onesd": _ONES,
        })
    return in_maps


def _run_device(x, Wq, Wk, Wv, Wo, trace=False):
    nc = _get_nc()
    in_maps = _make_in_maps(x, Wq, Wk, Wv, Wo)
    res = run_bass_kernel_spmd(nc, in_maps, core_ids=list(range(N_CORES)), trace=trace)
    if trace:
        return res
    return res.results


def run_traced(x, Wq, Wk, Wv, Wo):
    """Run with NTFF tracing; returns (full_output, BassKernelResults)."""
    res = _run_device(np.asarray(x, np.float32), np.asarray(Wq, np.float32),
                      np.asarray(Wk, np.float32), np.asarray(Wv, np.float32),
                      np.asarray(Wo, np.float32), trace=True)
    out = np.empty((B, S, D), dtype=np.float32)
    for b in range(B):
        acc = res.results[b * CORES_PER_BATCH]["out"].astype(np.float32)
        for i in range(1, CORES_PER_BATCH):
            acc = acc + res.results[b * CORES_PER_BATCH + i]["out"].astype(np.float32)
        out[b] = acc
    return out, res
